# revision 1
# baseline (speedup 1.0000x reference)
"""MultiHeadExternalAttention Trainium2 kernel.

Math (exact algebraic refactor of the reference):
  h = x @ W_in + b_in feeds ONLY the mk projection, and the mv/out_proj pair
  is linear in attn.  So fold on the host (float64):
    logits = x @ (W_in_h @ W_mk) + (b_in_h @ W_mk + b_mk)    -> K=512, M=256
    y = attnL1_all[n,256] @ V[256,512] + b_y                 -> K=256, N=512
  where V = stack_h(W_mv @ W_out_h), b_y = b_out + tile(b_mv) @ W_out.

Softmax over n runs in the transposed layout [m(+head) partitions, n free]:
exp with fused bias + per-row sum on ScalarE (accum_out); the L1 denominator
s[g,n] = sum_m exp/D via a masked matmul on the PE (contraction over
partitions, M=48 so a duplicate of s lands at psum rows 32-47); broadcast
back with constant mask matmuls whose lhsT sit at base partitions 0/32 —
the K=16 pair runs concurrently in separate PE row groups on hardware;
both scales fused into one VectorE scalar_tensor_tensor.  b_y is added on
the host.

GEMM1 runs in bf16 (x and W_comb are bf16 -> halves input DMA, measured
~1.8e-4 end-to-end rel err); GEMM2 and the mask matmuls run as float32r
(fp32 with 11-bit mantissa, full PE rate at N>=256), inputs pre-rounded.

Schedule: software pipeline, skew 1:
  iter i:  colsum(i-1) | GEMM1(i) slice | outer(i-1) | GEMM1(i) slice |
           first 2 y tiles of GEMM2(i-1) | GEMM1(i) rest | GEMM2(i-1) rest
           (per-n-tile y DMAs) | prefetch x(i+1)
PSUM budget (8 banks): attn chunks [128,512]x3 + s [16,512]x1 +
outer [128,512]x2 + y [128,512]x2.

Sharding: pure data-parallel over batch, 4 batches per core, 8 cores,
no collectives.
"""

import numpy as np

B, N, E = 32, 1024, 512
H, HD, M = 16, 128, 16
NCORES = 8
BPC = B // NCORES  # batches per core

# packed small-constant column offsets (see _pack_small)
_BC0 = 0          # bc: [128, 2]
_MK0 = 2          # mask: [128, 2, 48] -> cols 2 + 48t + g  (cols 32-47 dup 0-15)
_MT0 = 98         # maskT: rows 32t..32t+15, cols 98 + 128t + p
_WS_COLS = 354


def round_f32r(a):
    """Round float32 array to float32r (11-bit mantissa, RNE)."""
    a = np.ascontiguousarray(a, dtype=np.float32)
    u = a.view(np.uint32)
    lsb = (u >> 12) & 1
    u2 = (u + 0x7FF + lsb) & np.uint32(0xFFFFF000)
    return u2.view(np.float32)


_nc_cache = {}


def _build_program(repeat=1, interleave=False, ygroup=1, ydma="gpsimd",
                   ws_split=True):
    key = (repeat, interleave, ygroup, ydma, ws_split)
    if key in _nc_cache:
        return _nc_cache[key]
    import concourse.tile as tile
    from concourse import bacc, mybir

    f32 = mybir.dt.float32
    f32r = mybir.dt.float32r
    bf16 = mybir.dt.bfloat16
    Exp = mybir.ActivationFunctionType.Exp
    mult = mybir.AluOpType.mult

    nc = bacc.Bacc("TRN2", target_bir_lowering=False, debug=False)

    xt = nc.dram_tensor("xt", [BPC, 512, 1024], bf16, kind="ExternalInput").ap()
    wc = nc.dram_tensor("wc", [128, 2, 4, 128], bf16, kind="ExternalInput").ap()
    vv = nc.dram_tensor("vv", [128, 2, 512], f32r, kind="ExternalInput").ap()
    ws = nc.dram_tensor("ws", [128, _WS_COLS], f32r, kind="ExternalInput").ap()
    y = nc.dram_tensor("y", [BPC, 1024, 512], f32, kind="ExternalOutput").ap()

    NB = BPC * repeat  # logical batches (repeat only for benchmarking)

    with tile.TileContext(nc) as tc:
        with (
            tc.tile_pool(name="singles", bufs=1) as singles,
            tc.tile_pool(name="xt0p", bufs=4) as xt0p,
            tc.tile_pool(name="xtp", bufs=2) as xtp,
            tc.tile_pool(name="expp", bufs=8) as expp,
            tc.tile_pool(name="attnfp", bufs=8) as attnfp,
            tc.tile_pool(name="yp", bufs=8) as yp,
            tc.tile_pool(name="smallp", bufs=16) as smallp,
            tc.tile_pool(name="rsp", bufs=2) as rsp,
            tc.tile_pool(name="ps_attn", bufs=3, space="PSUM") as ps_attnp,
            tc.tile_pool(name="ps_s", bufs=1, space="PSUM") as ps_sp,
            tc.tile_pool(name="ps_outer", bufs=2, space="PSUM") as ps_outerp,
            tc.tile_pool(name="ps_y", bufs=2, space="PSUM") as ps_yp,
            nc.allow_low_precision(reason="f32r matmul operand chain"),
        ):
            # ---- preload the exp table set on ACT while DMAs stream ----
            dummy = smallp.tile([128, 1], f32, tag="dummy")
            nc.vector.memset(dummy, 0.0)
            dummy2 = smallp.tile([128, 1], f32, tag="dummy2")
            nc.scalar.activation(
                out=dummy2, in_=dummy, func=Exp, bias=0.0, scale=1.0
            )

            # ---- wc first (GEMM1 needs it), then x(0) ----
            wc_sb = singles.tile([128, 2, 4, 128], bf16, tag="wc")
            ws_sb = singles.tile([128, _WS_COLS], f32r, tag="ws")
            nc.sync.dma_start(out=wc_sb, in_=wc)
            if ws_split:
                # bc+mask columns land before x(0) so the first exp isn't
                # gated on the big constant load
                nc.sync.dma_start(out=ws_sb[:, 0:_MT0], in_=ws[:, 0:_MT0])

            # ---- pipeline state ----
            xts = {}     # i -> list of (tile, k-slicer)
            exps = {}    # i -> {(t, c): expT tile [128, 512]}
            rds = {}     # i -> [recipD_t0, recipD_t1] ([128, 1])
            lcss = {}    # i -> [lcs_t0, lcs_t1] ([128, 16])
            rss = {}     # i -> recipS [16, 1024] SBUF
            attnfs = {}  # i -> {(t, c): attnf tile [128, 512]}

            def load_x0():
                src = xt[0].rearrange("(k p) n -> p k n", p=128)
                tiles = []
                for k in range(4):
                    t = xt0p.tile([128, 1024], bf16, tag="xt0", name="xt0")
                    nc.sync.dma_start(out=t, in_=src[:, k, :])
                    tiles.append(t)
                xts[0] = tiles

            def load_x(i):
                t = xtp.tile([128, 4, 1024], bf16, tag="xt")
                nc.sync.dma_start(
                    out=t, in_=xt[i % BPC].rearrange("(k p) n -> p k n", p=128)
                )
                xts[i] = t

            def xt_rhs(i, k, c):
                if i == 0:
                    return xts[0][k][:, 512 * c : 512 * (c + 1)]
                return xts[i][:, k, 512 * c : 512 * (c + 1)]

            def gemm1_steps(i):
                """Yields 4 times; each step emits half of one t's GEMM1."""
                exps[i] = {}
                rds[i] = []
                lcss[i] = []
                for t in range(2):
                    # k-major so each weight tile is loaded once for both
                    # 512-column chunks (halves LDWEIGHTS traffic).  Batch 0
                    # is latency-critical: c-major there so the first chunk
                    # finishes (and frees its PSUM slot) 3 matmuls earlier.
                    pa = [
                        ps_attnp.tile([128, 512], f32, tag="attn", name="pa")
                        for _ in range(2)
                    ]
                    order = (
                        [(k, c) for c in range(2) for k in range(4)]
                        if i == 0
                        else [(k, c) for k in range(4) for c in range(2)]
                    )
                    for k, c in order[:4]:
                        nc.tensor.matmul(
                            pa[c],
                            lhsT=wc_sb[:, t, k, :],
                            rhs=xt_rhs(i, k, c),
                            start=(k == 0),
                            stop=(i == 0 and k == 3),
                        )
                    yield
                    for k, c in order[4:]:
                        nc.tensor.matmul(
                            pa[c],
                            lhsT=wc_sb[:, t, k, :],
                            rhs=xt_rhs(i, k, c),
                            start=(i == 0 and k == 0),
                            stop=(k == 3),
                        )
                    Dp = [None, None]
                    for c in range(2):
                        expT = expp.tile([128, 512], f32r, tag="exp", name="expT")
                        Dp[c] = smallp.tile([128, 1], f32, tag="Dp", name="Dp")
                        nc.scalar.activation(
                            out=expT,
                            in_=pa[c],
                            func=Exp,
                            bias=bc_ap(t),
                            scale=1.0,
                            accum_out=Dp[c],
                        )
                        exps[i][(t, c)] = expT
                    D = smallp.tile([128, 1], f32, tag="D")
                    nc.vector.tensor_add(D, Dp[0], Dp[1])
                    recipD = smallp.tile([128, 1], f32, tag="rD")
                    nc.vector.reciprocal(recipD, D)
                    lcs = smallp.tile([128, 48], f32r, tag="lcs")
                    nc.vector.tensor_scalar_mul(lcs, mask_ap(t), recipD)
                    rds[i].append(recipD)
                    lcss[i].append(lcs)
                    yield
                del xts[i]

            def colsum(i):
                rs = rsp.tile([48, 1024], f32r, tag="rs")
                for c in range(2):
                    ps_s = ps_sp.tile([48, 512], f32, tag="s")
                    for t in range(2):
                        nc.tensor.matmul(
                            ps_s,
                            lhsT=lcss[i][t],
                            rhs=exps[i][(t, c)],
                            start=(t == 0),
                            stop=(t == 1),
                        )
                    nc.vector.reciprocal(rs[:, 512 * c : 512 * (c + 1)], ps_s)
                rss[i] = rs

            def outer(i):
                attnfs[i] = {}
                for c in range(2):
                    for t in range(2):
                        po = ps_outerp.tile([128, 512], f32, tag="outer")
                        nc.tensor.matmul(
                            po,
                            lhsT=maskT_ap(t),
                            rhs=rss[i][32 * t : 32 * t + 16,
                                       512 * c : 512 * (c + 1)],
                            start=True,
                            stop=True,
                        )
                        attnf = attnfp.tile(
                            [128, 512], f32r, tag="attnf", name="attnf"
                        )
                        # attnf = (exp * 1/D) * outer
                        nc.vector.scalar_tensor_tensor(
                            out=attnf,
                            in0=exps[i][(t, c)],
                            scalar=rds[i][t],
                            in1=po,
                            op0=mult,
                            op1=mult,
                        )
                        attnfs[i][(t, c)] = attnf
                del rss[i]

            def ydma_start(out, in_, _n=[0]):
                _n[0] += 1
                if ydma == "split":
                    eng = nc.gpsimd if _n[0] % 2 else nc.sync
                elif ydma == "split2":
                    # alternate SWDGE (Pool) and the 2nd HWDGE ring (ACT);
                    # inputs keep the SP ring to themselves
                    eng = nc.gpsimd if _n[0] % 2 else nc.scalar
                elif ydma == "split3":
                    # asymmetric 6/2 split: sync also carries the inputs, so
                    # give it only a quarter of the output stream
                    eng = nc.gpsimd if (_n[0] - 1) % 8 < 6 else nc.sync
                elif ydma == "gpsimd":
                    eng = nc.gpsimd
                else:
                    eng = nc.sync
                return eng.dma_start(out=out, in_=in_)

            def gemm2_steps(i):
                """Yields after each n-tile group (group -> one DMA)."""
                for g in range(8 // ygroup):
                    yg = yp.tile([128, ygroup, 512], f32, tag="yt")
                    for j in range(ygroup):
                        ni = ygroup * g + j
                        c, col = divmod(ni, 4)
                        ps_out = ps_yp.tile([128, 512], f32, tag="y")
                        nc.tensor.matmul(
                            ps_out,
                            lhsT=attnfs[i][(0, c)][:, 128 * col : 128 * (col + 1)],
                            rhs=vv_sb[:, 0, :],
                            start=True,
                            stop=False,
                        )
                        nc.tensor.matmul(
                            ps_out,
                            lhsT=attnfs[i][(1, c)][:, 128 * col : 128 * (col + 1)],
                            rhs=vv_sb[:, 1, :],
                            start=False,
                            stop=True,
                        )
                        # evict (b_y is added on the host after gather)
                        nc.any.tensor_copy(yg[:, j, :], ps_out)
                    dst = y[
                        i % BPC, 128 * ygroup * g : 128 * ygroup * (g + 1), :
                    ].rearrange("(j p) e -> p j e", p=128)
                    # SWDGE (Pool) queue keeps the output stream independent
                    # of the input stream on SP (no head-of-line blocking)
                    ydma_start(out=dst, in_=yg)
                    yield
                del exps[i], rds[i], lcss[i], attnfs[i]

            def drain(gen):
                if gen is not None:
                    for _ in gen:
                        pass

            # ---- startup: wc, x(0), x(1), then remaining weights ----
            load_x0()
            if NB > 1:
                load_x(1)
            if ws_split:
                nc.sync.dma_start(
                    out=ws_sb[:, _MT0:_WS_COLS], in_=ws[:, _MT0:_WS_COLS]
                )
            else:
                nc.sync.dma_start(out=ws_sb, in_=ws)

            def bc_ap(t):
                return ws_sb[:, _BC0 + t : _BC0 + t + 1]

            def mask_ap(t):
                return ws_sb[:, _MK0 + 48 * t : _MK0 + 48 * (t + 1)]

            def maskT_ap(t):
                # base partition 32t -> the two outer matmuls of a chunk land
                # in different PE row groups and execute concurrently
                return ws_sb[32 * t : 32 * t + 16,
                             _MT0 + 128 * t : _MT0 + 128 * (t + 1)]

            vv_sb = singles.tile([128, 2, 512], f32r, tag="vv")
            nc.sync.dma_start(out=vv_sb, in_=vv)

            # ---- software pipeline (skew 1), interleaved ----
            # PE order is emission order.  Per iteration: colsum(i-1) first,
            # a GEMM1(i) slice to hide recipS latency, then outer(i-1), then
            # GEMM1 slices zipped with GEMM2(i-1) groups so y output flows
            # evenly across the iteration.
            for i in range(NB + 1):
                g1 = gemm1_steps(i) if i < NB else None
                g2 = gemm2_steps(i - 1) if 1 <= i else None
                if 1 <= i <= NB:
                    colsum(i - 1)
                if g1 is not None:
                    next(g1, None)
                if 1 <= i <= NB:
                    outer(i - 1)
                if g1 is not None:
                    next(g1, None)
                # hoist the first two y tiles of batch i-1 ahead of the rest
                # of GEMM1(i): the output stream restarts ~1.5us earlier
                for _ in range(2):
                    if g2 is not None:
                        next(g2, None)
                drain(g1)
                drain(g2)
                if 1 <= i + 1 < NB and i > 0:
                    load_x(i + 1)
    nc.compile()
    _nc_cache[key] = nc
    return nc


def _fold_weights(W_in, b_in, W_mk, b_mk, W_mv, b_mv, W_out, b_out):
    f64 = np.float64
    W_in_r = W_in.astype(f64).reshape(E, H, HD)          # [e, h, d]
    W_out_r = W_out.astype(f64).reshape(H, HD, E)        # [h, d, e]
    Wmk = W_mk.astype(f64)                               # [d, m]
    Wmv = W_mv.astype(f64)                               # [m, d]

    comb = np.einsum("ehd,dm->ehm", W_in_r, Wmk)         # [e, h, m]
    Wcg = comb.reshape(E, 2, 8 * M)                      # [e, t, c]
    # wc_host[p, t, k, c] = Wcg[128k + p, t, c]  (lhsT tile for (t, k))
    wc_host = np.ascontiguousarray(
        Wcg.reshape(4, 128, 2, 128).transpose(1, 2, 0, 3)
    ).astype(np.float32)

    bcomb = np.einsum("hd,dm->hm", b_in.astype(f64).reshape(H, HD), Wmk) + b_mk.astype(f64)
    bc_host = np.ascontiguousarray(bcomb.reshape(2, 128).T).astype(np.float32)  # [p, t]

    Vfull = np.einsum("md,hde->hme", Wmv, W_out_r)       # [h, m, e]
    vv_host = np.ascontiguousarray(
        Vfull.reshape(2, 128, E).transpose(1, 0, 2)
    ).astype(np.float32)                                 # [p, t, e]

    by_host = (
        b_out.astype(f64) + np.einsum("d,hde->e", b_mv.astype(f64), W_out_r)
    ).reshape(1, E).astype(np.float32)

    p = np.arange(128)
    g = np.arange(16)
    mask_host = np.zeros((128, 2, 16), np.float32)
    for t in range(2):
        mask_host[p, t, :] = (g[None, :] == (8 * t + p[:, None] // 16)).astype(np.float32)
    maskT_host = np.ascontiguousarray(mask_host.transpose(2, 1, 0))  # [g, t, p]

    ones_host = np.ones((1, 128), np.float32)
    return wc_host, bc_host, vv_host, by_host, mask_host, maskT_host, ones_host


def _pack_small(bc_h, mask_h, maskT_h, by_h, ones_h):
    ws = np.zeros((128, _WS_COLS), np.float32)
    ws[:, _BC0 : _BC0 + 2] = bc_h
    for t in range(2):
        # cols 0-15: head-index mask; cols 32-47: duplicate (produces a
        # second copy of s at psum rows 32-47 for the packed outer matmul)
        ws[:, _MK0 + 48 * t : _MK0 + 48 * t + 16] = mask_h[:, t, :]
        # cols 16-31 are never read downstream but keep them equal to the
        # mask so the reciprocal of those psum rows stays finite
        ws[:, _MK0 + 48 * t + 16 : _MK0 + 48 * t + 32] = mask_h[:, t, :]
        ws[:, _MK0 + 48 * t + 32 : _MK0 + 48 * (t + 1)] = mask_h[:, t, :]
        ws[32 * t : 32 * t + 16, _MT0 + 128 * t : _MT0 + 128 * (t + 1)] = (
            maskT_h[:, t, :]
        )
    return ws


def build_in_maps(x, W_in, b_in, W_mk, b_mk, W_mv, b_mv, W_out, b_out):
    wc_h, bc_h, vv_h, by_h, mask_h, maskT_h, ones_h = _fold_weights(
        W_in, b_in, W_mk, b_mk, W_mv, b_mv, W_out, b_out
    )
    import ml_dtypes

    # x [B, N, E] -> x^T per batch [B, E, N], bf16 (GEMM1 runs in bf16)
    xt_all = np.ascontiguousarray(
        np.asarray(x, dtype=np.float32).transpose(0, 2, 1)
    ).astype(ml_dtypes.bfloat16)
    wc_h = wc_h.astype(ml_dtypes.bfloat16)
    vv_h = round_f32r(vv_h)
    ws_h = _pack_small(bc_h, mask_h, maskT_h, by_h, ones_h)

    in_maps = []
    for c in range(NCORES):
        in_maps.append(
            {
                "xt": xt_all[BPC * c : BPC * (c + 1)],
                "wc": wc_h,
                "vv": vv_h,
                "ws": ws_h,
            }
        )
    return in_maps, by_h


def kernel(x, W_in, b_in, W_mk, b_mk, W_mv, b_mv, W_out, b_out):
    from concourse.bass_utils import run_bass_kernel_spmd

    # accept jax arrays or numpy
    x, W_in, b_in, W_mk, b_mk, W_mv, b_mv, W_out, b_out = (
        np.asarray(a)
        for a in (x, W_in, b_in, W_mk, b_mk, W_mv, b_mv, W_out, b_out)
    )
    in_maps, by_h = build_in_maps(
        x, W_in, b_in, W_mk, b_mk, W_mv, b_mv, W_out, b_out
    )
    nc = _build_program()

    res = run_bass_kernel_spmd(nc, in_maps, list(range(NCORES)))
    global _last_results
    _last_results = res
    out = np.concatenate([res.results[c]["y"] for c in range(NCORES)], axis=0)
    out += by_h  # b_y folded on the host
    return out


_last_results = None



# revision 36
# speedup vs baseline: 1.4813x; 1.4813x over previous
"""MultiHeadExternalAttention Trainium2 kernel (fp8 DoubleRow pipeline).

Math (exact algebraic refactor of the reference):
  h = x @ W_in + b_in feeds ONLY the mk projection, and the mv/out_proj pair
  is linear in attn.  Fold on the host (float64):
    logits = x @ (W_in_h @ W_mk) + (b_in_h @ W_mk + b_mk)    -> K=512, M=256
    y = attnL1_all[n,256] @ V[256,512] + b_y                 -> K=256, N=512
  where V = stack_h(W_mv @ W_out_h), b_y = b_out + tile(b_mv) @ W_out.

Precision/scale plan (end-to-end rel err ~8.9e-3 in numpy emulation, gate 2e-2):
  GEMM1: fp8e4m3 DoubleRow (x fp8, W_comb*64 fp8) -> psum = 64*logits;
         exp = Exp(psum/64 + bc) on ACT (scale folds the 64 back out),
         f32r out, accum_out gives the softmax denominator D pre-quantization.
  L1 norm: lcs = (mask/16)*rd f32r; s_psum = lcs^T exp (f32r matmul) = s/16;
         rs = 1/s_psum = 16/s (DVE reciprocal, bf16); po = broadcast of rs to
         128 partitions via a stride-0 SBUF->SBUF DMA (replaces the old PE
         "outer" matmuls); attnf = fp8(exp * rd * po) = 16*attn on GpSimd
         (all-SBUF - GPSIMD can't touch PSUM).
  GEMM2: fp8 DoubleRow folding both head-halves in one matmul:
         lhsT = attnf[:, :, 128j:128(j+1)], rhs = vv*64 fp8 -> y_psum = 1024*y.
  y: evicted psum->SBUF bf16 on ACT/DVE, DMA'd bf16; host does y/1024 + b_y.

Cost-model facts this design exploits (CoreSim V1 is the timing source here):
  - matmul cost = out_free_size * pe_cycle * cycles_per_row; K and M are free;
    fp8 DoubleRow = 0.5 cycles/row; f32r = 1.0 at N>=256.  GEMM1 drops 4x
    (k-pairs fold + double pump), GEMM2 drops 4x (t-fold + double pump).
  - DMA cost = free-dim bytes * 0.3855ns (partition dim uncharged), on the
    issuing engine's timeline; stride-0 src dims make broadcasts ~free.
  - engine op cost = free-size * cycle (+psum/sbuf access cycles on ACT/DVE);
    GPSIMD is cheapest (no access-cycle errata) but SBUF-only.

Sharding: pure data-parallel over batch, 4 batches per core, 8 cores.
"""

import numpy as np

B, N, E = 32, 1024, 512
H, HD, M = 16, 128, 16
NCORES = 8
BPC = B // NCORES  # batches per core

# ws column layout: bc [128, 2] then mask2 [128, 2, 16] (= mask/16)
_BC0 = 0
_MK0 = 2
_WS_COLS = 34

Y_SCALE = 1024.0  # attnf carries 16x, vv carries 64x


def round_f32r(a):
    """Round float32 array to float32r (11-bit mantissa, RNE)."""
    a = np.ascontiguousarray(a, dtype=np.float32)
    u = a.view(np.uint32)
    lsb = (u >> 12) & 1
    u2 = (u + 0x7FF + lsb) & np.uint32(0xFFFFF000)
    return u2.view(np.float32)


_nc_cache = {}


def _build_program(evict_acts=(0, 1, 2, 4, 5, 6, 7),
                   ydma_split=("gpsimd", "sync", "gpsimd", "sync"),
                   bcast_eng=("sync", "gpsimd"), ps_banks=(2, 1, 3),
                   x0_split=True):
    key = (evict_acts, ydma_split, bcast_eng, ps_banks, x0_split)
    if key in _nc_cache:
        return _nc_cache[key]
    import concourse.tile as tile
    from concourse import bacc, mybir

    f32 = mybir.dt.float32
    f32r = mybir.dt.float32r
    f8 = mybir.dt.float8e4
    bf16 = mybir.dt.bfloat16
    DR = mybir.MatmulPerfMode.DoubleRow
    Exp = mybir.ActivationFunctionType.Exp
    Copy = mybir.ActivationFunctionType.Copy
    mult = mybir.AluOpType.mult

    nc = bacc.Bacc("TRN2", target_bir_lowering=False, debug=False)

    xt = nc.dram_tensor("xt", [BPC, 512, 1024], f8, kind="ExternalInput").ap()
    wc = nc.dram_tensor("wc", [128, 2, 4, 128], f8, kind="ExternalInput").ap()
    vv = nc.dram_tensor("vv", [128, 2, 512], f8, kind="ExternalInput").ap()
    ws = nc.dram_tensor("ws", [128, _WS_COLS], f32, kind="ExternalInput").ap()
    mt = nc.dram_tensor("mt", [16, 2, 128], bf16, kind="ExternalInput").ap()
    y = nc.dram_tensor("y", [BPC, 1024, 512], bf16, kind="ExternalOutput").ap()

    NB = BPC

    with tile.TileContext(nc) as tc:
        with (
            tc.tile_pool(name="singles", bufs=1) as singles,
            tc.tile_pool(name="xtp", bufs=3) as xtp,
            tc.tile_pool(name="expp", bufs=3) as expp,
            tc.tile_pool(name="attnfp", bufs=3) as attnfp,
            tc.tile_pool(name="pop", bufs=8) as pop,
            tc.tile_pool(name="rsp", bufs=2) as rsp,
            tc.tile_pool(name="ygp", bufs=8) as ygp,
            tc.tile_pool(name="smallp", bufs=24) as smallp,
            tc.tile_pool(name="ps_pa", bufs=ps_banks[0], space="PSUM") as ps_pap,
            tc.tile_pool(name="ps_s", bufs=ps_banks[1], space="PSUM") as ps_sp,
            tc.tile_pool(name="ps_y", bufs=ps_banks[2], space="PSUM") as ps_yp,
            nc.allow_low_precision(reason="fp8 matmul operand chain"),
        ):
            # preload the exp table on ACT while the first DMAs stream
            dummy = smallp.tile([128, 1], f32, tag="dummy")
            nc.vector.memset(dummy, 0.0)
            dummy2 = smallp.tile([128, 1], f32, tag="dummy2")
            nc.scalar.activation(out=dummy2, in_=dummy, func=Exp, bias=0.0,
                                 scale=1.0)

            # weights + small constants first (GEMM1 gate), then x(0)
            wc_sb = singles.tile([128, 2, 4, 128], f8, tag="wc")
            ws_sb = singles.tile([128, _WS_COLS], f32, tag="ws")
            vv_sb = singles.tile([128, 2, 512], f8, tag="vv")
            mt_sb = singles.tile([16, 2, 128], bf16, tag="mt")


            def bc_ap(t):
                return ws_sb[:, _BC0 + t : _BC0 + t + 1]

            def mask2_ap(t):
                return ws_sb[:, _MK0 + 16 * t : _MK0 + 16 * (t + 1)]

            xts = {}     # i -> x tile [128, 4, 1024] f8
            exps = {}    # i -> [128, 2, 1024] f32r
            rds = {}     # i -> [rd_t0, rd_t1] [128, 1] f32
            lcss = {}    # i -> [128, 2, 16] f32r
            rss = {}     # i -> [16, 1024] bf16
            attnfs = {}  # (i, c) -> [128, 2, 512] f8
            pos = {}     # (i, t, c) -> [128, 512] bf16

            def load_x(i, eng=None, split=False):
                t = xtp.tile([128, 4, 1024], f8, tag="xt")
                src = xt[i].rearrange("(k p) n -> p k n", p=128)
                if split:
                    # two half-loads so the first k-pair lands sooner
                    nc.sync.dma_start(out=t[:, 0:2, :], in_=src[:, 0:2, :])
                    nc.sync.dma_start(out=t[:, 2:4, :], in_=src[:, 2:4, :])
                else:
                    (eng or nc.sync).dma_start(out=t, in_=src)
                xts[i] = t

            def g1_exp_t(i, t):
                """GEMM1 (DoubleRow, k-pairs) + fused exp for one head-half."""
                pa = ps_pap.tile([128, 1024], f32, tag="pa", name="pa")
                for c in range(2):
                    for kp in range(2):
                        nc.tensor.matmul(
                            pa[:, 512 * c : 512 * (c + 1)],
                            lhsT=wc_sb[:, t, 2 * kp : 2 * kp + 2, :],
                            rhs=xts[i][:, 2 * kp : 2 * kp + 2,
                                       512 * c : 512 * (c + 1)],
                            start=(kp == 0),
                            stop=(kp == 1),
                            perf_mode=DR,
                        )
                if i not in exps:
                    exps[i] = expp.tile([128, 2, 1024], f32r, tag="exp",
                                        name="exp")
                    rds[i] = [None, None]
                Dp = smallp.tile([128, 1], f32, tag="Dp", name="Dp")
                nc.scalar.activation(
                    out=exps[i][:, t, :], in_=pa, func=Exp, bias=bc_ap(t),
                    scale=1.0 / 64, accum_out=Dp,
                )
                rd = smallp.tile([128, 1], f32, tag="rd", name="rd")
                with tc.high_priority():
                    nc.vector.reciprocal(rd, Dp)
                rds[i][t] = rd
                if i not in lcss:
                    lcss[i] = smallp.tile([128, 2, 16], f32r, tag="lcs",
                                          name="lcs")
                with tc.high_priority():
                    nc.vector.tensor_scalar_mul(lcss[i][:, t, :], mask2_ap(t), rd)

            def colsum_c(i, c):
                ps_s = ps_sp.tile([16, 512], f32, tag="s")
                for t in range(2):
                    nc.tensor.matmul(
                        ps_s,
                        lhsT=lcss[i][:, t, :],
                        rhs=exps[i][:, t, 512 * c : 512 * (c + 1)],
                        start=(t == 0),
                        stop=(t == 1),
                    )
                if i not in rss:
                    rss[i] = rsp.tile([16, 1024], bf16, tag="rs", name="rs")
                with tc.high_priority():
                    nc.vector.reciprocal(rss[i][:, 512 * c : 512 * (c + 1)], ps_s)

            def outer_tc(i, t, c, pool):
                """po via PE outer matmul into spare psum (fill/drain path:
                skips the bcast-DMA init latency)."""
                po = pool.tile([128, 512], f32, tag="y" if pool is ps_yp
                               else "pa", name="po_ps")
                nc.tensor.matmul(
                    po,
                    lhsT=mt_sb[:, t, :],
                    rhs=rss[i][:, 512 * c : 512 * (c + 1)],
                    start=True,
                    stop=True,
                )
                pos[(i, t, c)] = po

            def outer_pa_t(i, t):
                """Drain path: po for both c in one 2-bank pa tile."""
                po = ps_pap.tile([128, 1024], f32, tag="pa", name="po_pa")
                for c in range(2):
                    nc.tensor.matmul(
                        po[:, 512 * c : 512 * (c + 1)],
                        lhsT=mt_sb[:, t, :],
                        rhs=rss[i][:, 512 * c : 512 * (c + 1)],
                        start=True,
                        stop=True,
                    )
                    pos[(i, t, c)] = po[:, 512 * c : 512 * (c + 1)]

            def bcast_tc(i, t, c, eng):
                """po[p, n] = rs[8t + p//16, 512c+n] via stride-0 DMA."""
                po = pop.tile([128, 512], bf16, tag="po", name="po")
                src = rss[i][8 * t : 8 * t + 8, 512 * c : 512 * (c + 1)] \
                    .unsqueeze(1).broadcast_to([8, 16, 512])
                eng.dma_start(out=po, in_=src)
                pos[(i, t, c)] = po

            def attnf_tc(i, t, c):
                if i not in attnfs:
                    attnfs[i] = attnfp.tile([128, 2, 1024], f8,
                                            tag="attnf", name="attnf")
                with tc.high_priority():
                    nc.vector.scalar_tensor_tensor(
                        out=attnfs[i][:, t, 512 * c : 512 * (c + 1)],
                        in0=exps[i][:, t, 512 * c : 512 * (c + 1)],
                        scalar=rds[i][t],
                        in1=pos[(i, t, c)],
                        op0=mult,
                        op1=mult,
                    )

            def g2_tile(i, j):
                """One n-tile of GEMM2: DoubleRow folds both head-halves."""
                ps_out = ps_yp.tile([128, 512], f32, tag="y")
                nc.tensor.matmul(
                    ps_out,
                    lhsT=attnfs[i][:, :, 128 * j : 128 * (j + 1)],
                    rhs=vv_sb,
                    start=True,
                    stop=True,
                    perf_mode=DR,
                )
                return ps_out

            def evict_tile(i, j, ps_out, yg):
                if j in evict_acts:
                    nc.scalar.activation(out=yg, in_=ps_out, func=Copy,
                                         bias=0.0, scale=1.0)
                else:
                    nc.vector.tensor_copy(yg, ps_out)

            def ydma_group(i, g, yg, eng):
                dst = y[i, 256 * g : 256 * (g + 1), :] \
                    .rearrange("(j p) e -> p j e", p=128)
                eng.dma_start(out=dst, in_=yg)

            engs = {"gpsimd": nc.gpsimd, "sync": nc.sync, "scalar": nc.scalar}

            def g2_cleanup(i):
                del exps[i], rds[i], lcss[i], attnfs[i]
                for t in range(2):
                    for c in range(2):
                        del pos[(i, t, c)]

            def g2_steps(i):
                """Yields after each pair of n-tiles (one y DMA group)."""
                for g in range(4):
                    yg = ygp.tile([128, 2, 512], bf16, tag="yg", name="yg")
                    for jj in range(2):
                        j = 2 * g + jj
                        ps_out = g2_tile(i, j)
                        evict_tile(i, j, ps_out, yg[:, jj, :])
                    ydma_group(i, g, yg, engs[ydma_split[g % len(ydma_split)]])
                    yield
                g2_cleanup(i)

            def g2_half_drain(i, half):
                """Drain-mode GEMM2: single-tile y DMAs spread over 3 queues,
                evicts alternating ACT/DVE so both engines drain in parallel."""
                ddma = ["sync", "gpsimd", "scalar", "sync"]
                for jj in range(4):
                    j = 4 * half + jj
                    ps_out = g2_tile(i, j)
                    yg = ygp.tile([128, 512], bf16, tag="ygd", name="ygd")
                    if jj % 2 == 0:
                        nc.scalar.activation(out=yg, in_=ps_out, func=Copy,
                                             bias=0.0, scale=1.0)
                    else:
                        nc.vector.tensor_copy(yg, ps_out)
                    dst = y[i, 128 * j : 128 * (j + 1), :] \
                        .rearrange("(o p) e -> p o e", p=128)
                    engs[ddma[jj]].dma_start(out=dst, in_=yg)
                if half == 1:
                    g2_cleanup(i)

            def drain(gen):
                if gen is not None:
                    for _ in gen:
                        pass

            # startup loads: Pool carries wc+ws, SP carries x(0) quarters,
            # ACT (idle until the first exp) carries x(1) and vv
            nc.gpsimd.dma_start(out=wc_sb, in_=wc)
            nc.gpsimd.dma_start(out=ws_sb, in_=ws)
            nc.gpsimd.dma_start(out=mt_sb, in_=mt)
            load_x(0, split=x0_split)
            load_x(1, eng=nc.gpsimd)
            nc.gpsimd.dma_start(out=vv_sb, in_=vv)

            b_engs = [engs[e] for e in bcast_eng]

            # Software pipeline, skew 2 for GEMM2:
            #   iter i: norm(i-1) | G1+exp(i) | G2+evict+ydma(i-2)
            # PE order: colsum(i-1) c0,c1 -> G1(i) t0,t1 -> G2(i-2); this keeps
            # the DVE queue (rs before recipD) and ACT queue (exp before
            # evicts) aligned with data readiness.  The last batch's GEMM2 is
            # folded into its norm iteration (drain shrink): each c-half runs
            # right after its attnf pair, with single-tile y DMAs on 3 queues.
            # PE order inside an iteration: G1(i, t0) first (its pa bank frees
            # as soon as exp(i-1, t0) ran, and it feeds ACT asap), then the
            # norm of i-1, then G1(i, t1), then G2(i-2).  This keeps ACT
            # saturated from the fill onward instead of HOL-blocking batch i
            # behind batch i-1's norm chain.
            for i in range(NB + 1):
                last = (i == NB)
                g2 = g2_steps(i - 2) if 2 <= i else None
                if i < NB:
                    g1_exp_t(i, 0)
                if 1 <= i:
                    j = i - 1
                    if j == 0:
                        # fill: y banks are free, po via PE outer (no DMA wait)
                        colsum_c(j, 0)
                        outer_tc(j, 0, 0, ps_yp)
                        outer_tc(j, 1, 0, ps_yp)
                        attnf_tc(j, 0, 0)
                        attnf_tc(j, 1, 0)
                        colsum_c(j, 1)
                        outer_tc(j, 0, 1, ps_yp)
                        outer_tc(j, 1, 1, ps_yp)
                    elif j == NB - 1:
                        # drain: pa banks are free, po via PE outer
                        colsum_c(j, 0)
                        colsum_c(j, 1)
                        outer_pa_t(j, 0)
                        outer_pa_t(j, 1)
                        attnf_tc(j, 0, 0)
                        attnf_tc(j, 1, 0)
                    else:
                        colsum_c(j, 0)
                        bcast_tc(j, 0, 0, b_engs[0])
                        bcast_tc(j, 1, 0, b_engs[1])
                        attnf_tc(j, 0, 0)
                        attnf_tc(j, 1, 0)
                        colsum_c(j, 1)
                        bcast_tc(j, 0, 1, b_engs[0])
                        bcast_tc(j, 1, 1, b_engs[1])
                if i < NB:
                    g1_exp_t(i, 1)
                if g2 is not None:
                    next(g2, None)  # 2 G2 tiles + y DMA group 0
                if 1 <= i:
                    attnf_tc(i - 1, 0, 1)
                    attnf_tc(i - 1, 1, 1)
                drain(g2)
                if last:
                    g2_half_drain(NB - 1, 0)
                    g2_half_drain(NB - 1, 1)
                if i + 2 < NB:
                    load_x(i + 2)
    nc.compile()
    _nc_cache[key] = nc
    return nc


def _fold_weights(W_in, b_in, W_mk, b_mk, W_mv, b_mv, W_out, b_out):
    f64 = np.float64
    W_in_r = W_in.astype(f64).reshape(E, H, HD)          # [e, h, d]
    W_out_r = W_out.astype(f64).reshape(H, HD, E)        # [h, d, e]
    Wmk = W_mk.astype(f64)                               # [d, m]
    Wmv = W_mv.astype(f64)                               # [m, d]

    comb = np.einsum("ehd,dm->ehm", W_in_r, Wmk)         # [e, h, m]
    Wcg = comb.reshape(E, 2, 8 * M)                      # [e, t, c]
    # wc_host[p, t, k, c] = Wcg[128k + p, t, c]  (lhsT tile for (t, k))
    wc_host = np.ascontiguousarray(
        Wcg.reshape(4, 128, 2, 128).transpose(1, 2, 0, 3)
    ).astype(np.float32)

    bcomb = np.einsum("hd,dm->hm", b_in.astype(f64).reshape(H, HD), Wmk) + b_mk.astype(f64)
    bc_host = np.ascontiguousarray(bcomb.reshape(2, 128).T).astype(np.float32)  # [p, t]

    Vfull = np.einsum("md,hde->hme", Wmv, W_out_r)       # [h, m, e]
    vv_host = np.ascontiguousarray(
        Vfull.reshape(2, 128, E).transpose(1, 0, 2)
    ).astype(np.float32)                                 # [p, t, e]

    by_host = (
        b_out.astype(f64) + np.einsum("d,hde->e", b_mv.astype(f64), W_out_r)
    ).reshape(1, E).astype(np.float32)

    p = np.arange(128)
    g = np.arange(16)
    mask_host = np.zeros((128, 2, 16), np.float32)
    for t in range(2):
        mask_host[p, t, :] = (g[None, :] == (8 * t + p[:, None] // 16)).astype(np.float32)
    maskT_host = np.ascontiguousarray(mask_host.transpose(2, 1, 0))  # [g, t, p]

    ones_host = np.ones((1, 128), np.float32)
    return wc_host, bc_host, vv_host, by_host, mask_host, maskT_host, ones_host


def _pack_small(bc_h, mask_h):
    ws = np.zeros((128, _WS_COLS), np.float32)
    ws[:, _BC0 : _BC0 + 2] = bc_h
    for t in range(2):
        # mask2 = mask/16: makes rs = 16/s so attnf = 16*attn fits fp8 range
        ws[:, _MK0 + 16 * t : _MK0 + 16 * (t + 1)] = mask_h[:, t, :] / 16.0
    return ws


def build_in_maps(x, W_in, b_in, W_mk, b_mk, W_mv, b_mv, W_out, b_out):
    wc_h, bc_h, vv_h, by_h, mask_h, maskT_h, ones_h = _fold_weights(
        W_in, b_in, W_mk, b_mk, W_mv, b_mv, W_out, b_out
    )
    import ml_dtypes

    f8 = ml_dtypes.float8_e4m3
    # x [B, N, E] -> x^T per batch [B, E, N], fp8 e4m3
    xt_all = np.ascontiguousarray(
        np.asarray(x, dtype=np.float32).transpose(0, 2, 1)
    ).astype(f8)
    wc_q = (wc_h * 64.0).astype(f8)
    vv_q = (vv_h * 64.0).astype(f8)
    ws_h = _pack_small(bc_h, mask_h)
    mt_h = np.ascontiguousarray(maskT_h).astype(ml_dtypes.bfloat16)  # [g, t, p]

    in_maps = []
    for c in range(NCORES):
        in_maps.append(
            {
                "xt": xt_all[BPC * c : BPC * (c + 1)],
                "wc": wc_q,
                "vv": vv_q,
                "ws": ws_h,
                "mt": mt_h,
            }
        )
    return in_maps, by_h


def kernel(x, W_in, b_in, W_mk, b_mk, W_mv, b_mv, W_out, b_out):
    from concourse.bass_utils import run_bass_kernel_spmd

    x, W_in, b_in, W_mk, b_mk, W_mv, b_mv, W_out, b_out = (
        np.asarray(a)
        for a in (x, W_in, b_in, W_mk, b_mk, W_mv, b_mv, W_out, b_out)
    )
    in_maps, by_h = build_in_maps(
        x, W_in, b_in, W_mk, b_mk, W_mv, b_mv, W_out, b_out
    )
    nc = _build_program()

    res = run_bass_kernel_spmd(nc, in_maps, list(range(NCORES)))
    global _last_results
    _last_results = res
    out = np.concatenate(
        [res.results[c]["y"].astype(np.float32) for c in range(NCORES)], axis=0
    )
    out = out / Y_SCALE + by_h  # undo fp8 scales, add folded bias
    return out


_last_results = None
